# revision 22
# baseline (speedup 1.0000x reference)
"""GRU encoder (nn_Encoder_26087631356042) Bass/Trainium2 kernel, v2.

Data-parallel over batch (B=128 -> 16 rows/core on 8 cores, no collectives).
Everything is latency-bound by the 2048-step recurrence, so the design
minimizes the number of instructions on the per-step serial dependency
chain:

- All recurrent matmuls are exact fp32 (float32r): 12 weight MMs + 3
  "opener" MMs per step instead of 38 bf16 hi/lo MMs.
- The openers start each PSUM accumulation group AND inject the
  precomputed input projections (identity-matrix matmul), so the gate
  preactivations appear in PSUM complete: no separate "xp + hp" add.
- PSUM is split per gate (r / z / n tiles) so the sigmoid for r only
  waits on the 4 r matmuls, and the z-gate work runs in the tanh shadow.
- W_hh/b_hh for the n gate are prescaled by 0.5 on the host; with
  sigma(x) = (1+tanh(x/2))/2 this keeps the polynomial variant exact-form
  compatible (q = 1+p = 2*sigma).
- The input projection GEMM (bf16 hi/lo, 3-pass, exact to ~1e-6) for
  chunk c+1 is spread between the steps of chunk c, filling idle PE time.

Per-step critical path (engine: op):
  PE: 4 r-MMs -> Act: sigmoid(r) [PSUM->PSUM] -> DVE: t1=r*hn' ->
  DVE: w=t1+xn -> Act: tanh -> DVE: t3=(1-z)*n -> DVE: h'=t3+z*h
with sigmoid(z), sigmoid(-z) (Act) and z*h (DVE) scheduled in the shadow.
"""

import numpy as np
import ml_dtypes
from contextlib import ExitStack, contextmanager

import concourse.bass as bass
import concourse.bacc as bacc
import concourse.tile as tile
import concourse.mybir as mybir
from concourse.bass_utils import run_bass_kernel_spmd

F32 = mybir.dt.float32
F32R = mybir.dt.float32r
BF16 = mybir.dt.bfloat16
AF = mybir.ActivationFunctionType

B, T, X, H = 128, 2048, 128, 256
G = 3 * H          # 768 gate features
NBLK = G // 128    # 6 feature blocks (r: 0,1  z: 2,3  n: 4,5)
NCORES = 8
BL = B // NCORES   # 16 batch rows per core
CH = 64            # timesteps per chunk
P = 128
PAD_CH = 2         # extra zero chunks so the software pipeline can overrun

bf16 = ml_dtypes.bfloat16


def _split_hi_lo(a32: np.ndarray):
    hi = a32.astype(bf16)
    lo = (a32 - hi.astype(np.float32)).astype(bf16)
    return hi, lo


def _build_program(t_steps: int, reps: int = 1, unroll: bool = False):
    nchunks = t_steps // CH
    assert nchunks * CH == t_steps and nchunks % 2 == 0
    nbody = nchunks // 2
    nc = bacc.Bacc(
        "TRN2", target_bir_lowering=False, debug=False, num_devices=NCORES
    )

    xin_cols = (nchunks + PAD_CH) * CH * BL
    d_xin_hi = nc.dram_tensor("xin_hi", [P, xin_cols], BF16, kind="ExternalInput")
    d_xin_lo = nc.dram_tensor("xin_lo", [P, xin_cols], BF16, kind="ExternalInput")
    d_whh = nc.dram_tensor("whh", [P, 2 * G], F32, kind="ExternalInput")
    d_wih_hi = nc.dram_tensor("wih_hi", [P, G], BF16, kind="ExternalInput")
    d_wih_lo = nc.dram_tensor("wih_lo", [P, G], BF16, kind="ExternalInput")
    d_biases = nc.dram_tensor("biases", [P, NBLK], F32, kind="ExternalInput")
    d_ident = nc.dram_tensor("ident", [P, P], F32, kind="ExternalInput")
    d_bnb = nc.dram_tensor("bnb", [P, 2 * BL], F32, kind="ExternalInput")
    d_out = nc.dram_tensor("hout", [P, 2 * BL], F32, kind="ExternalOutput")

    with tile.TileContext(nc) as tc, ExitStack() as ctx:
        cpool = ctx.enter_context(tc.tile_pool(name="const", bufs=1))
        pstep = ctx.enter_context(tc.tile_pool(name="pstep", bufs=2, space="PSUM"))
        px = ctx.enter_context(tc.tile_pool(name="px", bufs=2, space="PSUM"))
        gsb = ctx.enter_context(tc.tile_pool(name="gates", bufs=2))

        # ---- constants in SBUF ----
        whh = cpool.tile([P, 2 * G], F32, tag="whh")
        wih_hi = cpool.tile([P, G], BF16, tag="wih_hi")
        wih_lo = cpool.tile([P, G], BF16, tag="wih_lo")
        biases = cpool.tile([P, NBLK], F32, tag="biases")
        ident = cpool.tile([P, P], F32, tag="ident")
        bnb = cpool.tile([P, 2 * BL], F32, tag="bnb")
        for dst, src in ((whh, d_whh), (wih_hi, d_wih_hi), (wih_lo, d_wih_lo),
                         (biases, d_biases), (ident, d_ident), (bnb, d_bnb)):
            nc.sync.dma_start(dst[:], src.ap()[:])
        whh_r = whh
        ident_r = ident
        bnb_r = bnb

        # xin chunk tiles (even chunks -> 0, odd -> 1)
        xin_hi = [cpool.tile([P, CH * BL], BF16, name=f"xih{i}", tag=f"xih{i}")
                  for i in range(2)]
        xin_lo = [cpool.tile([P, CH * BL], BF16, name=f"xil{i}", tag=f"xil{i}")
                  for i in range(2)]
        # xp slabs, one per chunk parity: [P, block, step*BL]
        slabs = [cpool.tile([P, NBLK, CH * BL], F32, name=f"xp{i}", tag=f"xp{i}")
                 for i in range(2)]
        slabs_r = slabs

        # hidden state ping-pong (step parity; CH even so chunks start at 0)
        hT = [cpool.tile([P, 2 * BL], F32, name=f"hT{j}", tag=f"hT{j}")
              for j in range(2)]
        hTr = hT
        nc.gpsimd.memset(hT[0][:], 0)

        def dma_chunk(par: int, chunk_i):
            cols = CH * BL
            nc.sync.dma_start(
                xin_hi[par][:], d_xin_hi.ap()[:, bass.ts(chunk_i, cols)])
            nc.sync.dma_start(
                xin_lo[par][:], d_xin_lo.ap()[:, bass.ts(chunk_i, cols)])

        def phase1_group(par: int, g: int):
            """One phase-1 unit: 3 MMs + 1 biased evacuation for (m, hf)."""
            m, hf = divmod(g, CH * BL // 512)
            pxm = px.tile([P, 512], F32, tag="pxm")
            wsl = slice(128 * m, 128 * (m + 1))
            xsl = slice(512 * hf, 512 * (hf + 1))
            xh, xl = xin_hi[par], xin_lo[par]
            nc.tensor.matmul(pxm[:], wih_hi[:, wsl], xh[:, xsl],
                             start=True, stop=False)
            nc.tensor.matmul(pxm[:], wih_hi[:, wsl], xl[:, xsl],
                             start=False, stop=False)
            nc.tensor.matmul(pxm[:], wih_lo[:, wsl], xh[:, xsl],
                             start=False, stop=True)
            nc.scalar.activation(slabs[par][:, m, xsl], pxm[:], AF.Identity,
                                 bias=biases[:, m:m + 1])

        NP1 = NBLK * (CH * BL // 512)        # phase-1 groups per chunk (12)
        p1_sched = {int(np.floor(g * CH / NP1)): g for g in range(NP1)}

        def alloc_ps():
            hpr = pstep.tile([P, 2 * BL], F32, tag="hpr")
            hpz = pstep.tile([P, 2 * BL], F32, tag="hpz")
            hpn = pstep.tile([P, 2 * BL], F32, tag="hpn")
            return hpr, hpz, hpn

        def emit_openers(ps, slab_par: int, s: int):
            """Open the psum groups for step s and inject xp / n-bias."""
            hpr, hpz, hpn = ps
            xpr = slabs_r[slab_par]
            bsl = slice(s * BL, (s + 1) * BL)
            nc.tensor.matmul(hpr[:], ident_r[:],
                             xpr[:, 0:2, bsl], start=True, stop=False)
            nc.tensor.matmul(hpz[:], ident_r[:],
                             xpr[:, 2:4, bsl], start=True, stop=False)
            nc.tensor.matmul(hpn[:], ident_r[:],
                             bnb_r[:], start=True, stop=False)

        def emit_step(s_par: int, ps, next_ps, slab_par: int,
                      next_slab_par: int, s: int, p1=None):
            """One GRU step. ps were opened previously; emits openers for the
            NEXT step and (optionally) a phase-1 group for the next chunk."""
            cur, nxt = s_par, 1 - s_par
            hpr, hpz, hpn = ps
            h_r = hTr[cur]

            # recurrent matmuls: r first (critical), then n, then z
            def mm(tgt, blk, k, last):
                wsl = slice(G * k + 128 * blk, G * k + 128 * (blk + 1))
                osl = slice((blk % 2) * BL, (blk % 2 + 1) * BL)
                nc.tensor.matmul(tgt[:, osl], whh_r[:, wsl],
                                 h_r[:, k * BL: (k + 1) * BL],
                                 start=False, stop=last)
            for k in range(2):
                mm(hpr, 0, k, False)
                mm(hpr, 1, k, k == 1)
            for k in range(2):
                mm(hpn, 4, k, False)
                mm(hpn, 5, k, k == 1)
            for k in range(2):
                mm(hpz, 2, k, False)
                mm(hpz, 3, k, k == 1)

            # next step's openers (run in PE idle time)
            emit_openers(next_ps, next_slab_par, (s + 1) % CH)

            # ---- elementwise ----
            r_ps = gsb.tile([P, 2 * BL], F32, tag="rps")
            t1 = gsb.tile([P, 2 * BL], F32, tag="t1")
            w = gsb.tile([P, 2 * BL], F32, tag="w")
            nsb = gsb.tile([P, 2 * BL], F32, tag="nsb")
            z = gsb.tile([P, 2 * BL], F32, tag="z")
            zc = gsb.tile([P, 2 * BL], F32, tag="zc")
            zh = gsb.tile([P, 2 * BL], F32, tag="zh")
            t3 = gsb.tile([P, 2 * BL], F32, tag="t3")

            xp_n = slabs[slab_par][:, 4:6, s * BL:(s + 1) * BL]

            # Act: r sigmoid (critical), then z / 1-z shadow, then tanh
            nc.scalar.activation(r_ps[:], hpr[:], AF.Sigmoid)
            nc.scalar.activation(z[:], hpz[:], AF.Sigmoid)
            nc.scalar.activation(zc[:], hpz[:], AF.Sigmoid, scale=-1.0)
            # DVE chain
            nc.vector.tensor_mul(t1[:], r_ps[:], hpn[:])
            nc.vector.tensor_add(w[:], t1[:], xp_n)
            # z*h runs while tanh is in flight
            nc.vector.tensor_mul(zh[:], z[:], hT[cur][:])
            nc.scalar.activation(nsb[:], w[:], AF.Tanh)
            nc.vector.tensor_mul(t3[:], zc[:], nsb[:])
            nc.vector.tensor_add(hT[nxt][:], t3[:], zh[:])

            # spread phase-1 for the next chunk; emitted last so the Act
            # evacuation queues behind this step's tanh
            if p1 is not None:
                phase1_group(*p1)

        @contextmanager
        def _unrolled_iter(i):
            yield i

        def loop_ctxs(n):
            if unroll:
                return [_unrolled_iter(i) for i in range(n)]
            return [tc.For_i(0, n,
                             hint_engines=(mybir.EngineType.PE,
                                           mybir.EngineType.DVE))]

        def emit_time_loop():
            # prologue: chunk 0 in, phase-1 for chunk 0, prefetch chunk 1
            dma_chunk(0, 0)
            for g in range(NP1):
                phase1_group(0, g)
            dma_chunk(1, 1)
            ps = alloc_ps()
            emit_openers(ps, 0, 0)

            state = {"ps": ps}

            def do_chunk(half, ci_next_dma):
                # chunk with parity `half`; DMA chunk ci_next_dma into
                # xin[half] (2 chunks ahead); phase-1 for the NEXT chunk
                # (parity 1-half) interleaved into the steps.
                dma_chunk(half, ci_next_dma)
                for s in range(CH):
                    nxt_slab = half if s < CH - 1 else 1 - half
                    next_ps = alloc_ps()
                    g = p1_sched.get(s)
                    emit_step(s % 2, state["ps"], next_ps, half, nxt_slab, s,
                              p1=(1 - half, g) if g is not None else None)
                    state["ps"] = next_ps

            def body(j):
                # j-th 2-chunk body: chunks 2j (parity 0) and 2j+1 (parity 1)
                do_chunk(0, 2 * j + 2)
                do_chunk(1, 2 * j + 3)

            for ctx_ in loop_ctxs(nbody):
                with ctx_ as j:
                    body(j)

        if reps > 1:
            with tc.For_i(0, reps, name="rep"):
                emit_time_loop()
        else:
            emit_time_loop()

        nc.sync.dma_start(d_out.ap()[:], hT[0][:])

    nc.compile()
    return nc


_PROGRAM_CACHE: dict = {}


def _get_program(t_steps: int, reps: int = 1):
    key = (t_steps, reps)
    if key not in _PROGRAM_CACHE:
        _PROGRAM_CACHE[key] = _build_program(t_steps, reps)
    return _PROGRAM_CACHE[key]


def _pack_inputs(input, W_ih, W_hh, b_ih, b_hh, t_steps: int):
    input = np.asarray(input, np.float32)
    W_ih = np.asarray(W_ih, np.float32)
    W_hh = np.asarray(W_hh, np.float32)
    b_ih = np.asarray(b_ih, np.float32)
    b_hh = np.asarray(b_hh, np.float32)

    # W_hh packed k-block-major
    whhT = np.ascontiguousarray(W_hh.T)              # [H, G]
    whh = whhT.reshape(2, P, G).transpose(1, 0, 2).reshape(P, 2 * G)
    whh = np.ascontiguousarray(whh).astype(np.float32)

    wihT = np.ascontiguousarray(W_ih.T)              # [X, G]
    wih_hi, wih_lo = _split_hi_lo(wihT)

    # phase-1 evacuation biases: r,z get b_ih+b_hh; n gets b_ih only
    bias_vec = b_ih.copy()
    bias_vec[: 2 * H] += b_hh[: 2 * H]
    biases = np.ascontiguousarray(bias_vec.reshape(NBLK, P).T).astype(np.float32)

    # n-gate recurrent bias, broadcast over batch
    bnb = b_hh[2 * H:].reshape(2, P)                 # [blk, p]
    bnb = np.ascontiguousarray(
        np.repeat(bnb.T[:, :, None], BL, axis=2).reshape(P, 2 * BL)
    ).astype(np.float32)

    ident = np.eye(P, dtype=np.float32)

    shared = dict(whh=whh, wih_hi=wih_hi, wih_lo=wih_lo, biases=biases,
                  ident=ident, bnb=bnb)

    pad_steps = PAD_CH * CH
    in_maps = []
    for c in range(NCORES):
        xs = input[c * BL: (c + 1) * BL, :t_steps, :]     # [16, t, 128]
        xt = np.ascontiguousarray(xs.transpose(2, 1, 0))  # [128, t, 16]
        xt = xt.reshape(P, t_steps * BL)
        xpad = np.zeros((P, (t_steps + pad_steps) * BL), np.float32)
        xpad[:, : t_steps * BL] = xt
        xh, xl = _split_hi_lo(xpad)
        m = dict(shared)
        m["xin_hi"] = xh
        m["xin_lo"] = xl
        in_maps.append(m)
    return in_maps


def _unpack_output(results):
    out = np.empty((B, H), np.float32)
    for c in range(NCORES):
        o = results[c]["hout"].reshape(P, 2, BL)           # [p, k, b]
        out[c * BL: (c + 1) * BL, :] = o.transpose(2, 1, 0).reshape(BL, H)
    return out


def run(input, W_ih, W_hh, b_ih, b_hh, t_steps: int = T, trace: bool = False):
    nc = _get_program(t_steps)
    in_maps = _pack_inputs(input, W_ih, W_hh, b_ih, b_hh, t_steps)
    res = run_bass_kernel_spmd(
        nc, in_maps, core_ids=list(range(NCORES)), trace=trace
    )
    return _unpack_output(res.results), res


def kernel(input, W_ih, W_hh, b_ih, b_hh):
    out, _ = run(input, W_ih, W_hh, b_ih, b_hh)
    return out


def bench(input, W_ih, W_hh, b_ih, b_hh, reps_hi: int = 49, iters: int = 4):
    """On-device time estimate: (wall(R) - wall(1)) / (R - 1).

    Runs the two executables alternately and takes min-of-iters for each to
    cancel the (large, drifting) host/transfer overhead."""
    import time as _time

    in_maps = _pack_inputs(input, W_ih, W_hh, b_ih, b_hh, T)
    nc1 = _get_program(T, 1)
    ncR = _get_program(T, reps_hi)

    def once(nc):
        t0 = _time.perf_counter()
        run_bass_kernel_spmd(nc, in_maps, core_ids=list(range(NCORES)))
        return _time.perf_counter() - t0

    once(nc1)
    once(ncR)
    s1, sR = [], []
    for _ in range(iters):
        s1.append(once(nc1))
        sR.append(once(ncR))
    t1, tR = min(s1), min(sR)
    ns = (tR - t1) / (reps_hi - 1) * 1e9
    print("samples R=1:", [f"{v*1e3:.0f}" for v in s1],
          f" R={reps_hi}:", [f"{v*1e3:.0f}" for v in sR])
    print(f"wall R=1: {t1*1e3:.1f} ms   wall R={reps_hi}: {tR*1e3:.1f} ms")
    return ns
